# revision 1
# baseline (speedup 1.0000x reference)
"""AttentionConv3D Trainium2 kernel.

Computation (per channel c, voxel (d,h,w)):
    q,k,v = 1x1x1 convs of x;  s_kv = q * (k_pad[nbr kv] + rel_bias(c,kv))
    out   = sum_kv softmax_kv(s) * v_pad[nbr kv]         (27 = 3x3x3 window)

Strategy: depth-shard over 8 cores (2 output depth planes each, 1-plane halo).
Host zero-pads x to [64,18,66,66] so the channel-mix matmuls directly produce
zero-padded k/v/q planes. On-device layout: partition p = dl*64 + c
(dl in {0,1} local depth), free dim = padded 66x66 plane (4356).
Per kv-neighbor the window access is a free-dim offset (kh-1)*66 + (kw-1) into
one of three depth-plane buffers K[kd]; the rel bias collapses to a
per-partition scalar B[p, kv], so s = (K_shift + B)*q is ONE DVE
scalar_tensor_tensor op. exp on ACT; numerator/denominator accumulated with
identity matmuls into PSUM on the TensorEngine; 1/den via exp(-ln(den)) on ACT.
"""

import sys
import numpy as np

for _p in ("/opt/trn_rl_repo", "/root/.axon_site/_ro/trn_rl_repo"):
    if _p not in sys.path:
        sys.path.insert(0, _p)

HP = 66               # padded plane edge
HW = HP * HP          # 4356
NPL = 4               # k/v depth planes per core (2 outputs + halo)
R0 = 67               # first interior padded-linear position
CHUNKS = [(67, 1402), (1469, 1536), (3005, 1284)]  # covers [67, 4289); chunk 0's
# window reads ([67-67, 67+67+1402) = [0,1536)) fit inside proj col-chunk 0 so
# the kv loop overlaps the tail of the projection phase.
PROJ = [(0, 1536), (1536, 1536), (3072, 1284)]     # proj psum chunks over 4356
OUT_ROWS = [(0, 21), (21, 44), (44, 64)]           # row bands DMA'd per chunk

# hot-path dtype knobs (fp32 = safe; bf16 halves DVE cost of the e*v path)
E_BF16 = True   # e / v / ev tiles + identity in bf16 (PE still accums fp32)

_CACHE = {}


def _subs(L):
    return [(0, 512), (512, 512), (1024, L - 1024)]


def _build():
    from contextlib import ExitStack
    import concourse.bacc as bacc
    import concourse.tile as tile
    from concourse import mybir

    f32 = mybir.dt.float32
    bf16 = mybir.dt.bfloat16
    edt = bf16 if E_BF16 else f32
    Alu = mybir.AluOpType
    Act = mybir.ActivationFunctionType

    nc = bacc.Bacc("TRN2", target_bir_lowering=False)
    xs_d = nc.dram_tensor("xs", [64, NPL * HW], f32, kind="ExternalInput")
    wk_d = nc.dram_tensor("wk2", [64, 128], f32, kind="ExternalInput")
    wv_d = nc.dram_tensor("wv2", [64, 128], f32, kind="ExternalInput")
    wq_d = nc.dram_tensor("wq2", [64, 128], f32, kind="ExternalInput")
    b_d = nc.dram_tensor("bias", [128, 27], f32, kind="ExternalInput")
    id_d = nc.dram_tensor("ident", [128, 128], edt, kind="ExternalInput")
    out_d = nc.dram_tensor("out", [128, 64, 64], f32, kind="ExternalOutput")

    with tile.TileContext(nc) as tc, ExitStack() as ctx:
        singles = ctx.enter_context(tc.tile_pool(name="singles", bufs=1))
        planes = ctx.enter_context(tc.tile_pool(name="planes", bufs=1))
        wpool = ctx.enter_context(tc.tile_pool(name="work", bufs=2))

        wk_s = singles.tile([64, 128], f32, tag="wk")
        wv_s = singles.tile([64, 128], f32, tag="wv")
        wq_s = singles.tile([64, 128], f32, tag="wq")
        id_s = singles.tile([128, 128], edt, tag="id")
        b_s = singles.tile([128, 27], f32, tag="b")
        ebias = singles.tile([128, 1], f32, tag="ebias")
        nc.vector.memset(ebias[:], -28.0)
        for t, d in ((wk_s, wk_d), (wv_s, wv_d), (wq_s, wq_d),
                     (id_s, id_d), (b_s, b_d)):
            nc.sync.dma_start(t[:], d[:])

        Kp = [planes.tile([128, HW], f32, tag=f"k{i}", name=f"k{i}") for i in range(3)]
        Vp = [planes.tile([128, HW], edt, tag=f"v{i}", name=f"v{i}") for i in range(3)]
        Q = planes.tile([128, HW], f32, tag="q")
        OUT = planes.tile([128, HW], f32, tag="o")

        # ---- projections: plane m of xs -> k/v (dual-copy weights give the
        # same output plane on partitions 0:64 and 64:128), q for m in {1,2}.
        # column-chunk OUTER so all planes' first 1536 columns (what kv chunk 0
        # needs) are projected before any plane's later columns.
        with tc.tile_pool(name="xp", bufs=3) as xpool, \
             tc.tile_pool(name="pp", bufs=2, space="PSUM") as ppool:
            for base, L3 in PROJ:
                for m in range(NPL):
                    X = xpool.tile([64, 1536], f32, tag="x")
                    nc.sync.dma_start(X[:, :L3],
                                      xs_d[:, m * HW + base:m * HW + base + L3])
                    projs = [(wk_s, "k"), (wv_s, "v")]
                    if m in (1, 2):
                        projs.append((wq_s, "q"))
                    for w_s, kind in projs:
                        pp = ppool.tile([128, 1536], f32, tag="pp")
                        for a, bl in _subs(L3):
                            nc.tensor.matmul(pp[:, a:a + bl], w_s[:],
                                             X[:, a:a + bl],
                                             start=True, stop=True)
                        sl = (slice(0, 64), slice(base, base + L3))
                        sh = (slice(64, 128), slice(base, base + L3))
                        if kind == "k":
                            # split k evacuations across DVE/ACT to keep DVE,
                            # the span-limiting engine, under ACT's load
                            if m <= 2:
                                nc.vector.tensor_copy(Kp[m][sl], pp[0:64, :L3])
                            if m >= 1:
                                nc.scalar.copy(Kp[m - 1][sh], pp[64:128, :L3])
                        elif kind == "v":
                            if m <= 2:
                                nc.scalar.copy(Vp[m][sl], pp[0:64, :L3])
                            if m >= 1:
                                nc.scalar.copy(Vp[m - 1][sh], pp[64:128, :L3])
                        elif m == 1:
                            nc.vector.tensor_copy(Q[sl], pp[0:64, :L3])
                        else:
                            nc.scalar.copy(Q[sh], pp[64:128, :L3])

        # ---- 27-neighbor softmax attention, PSUM-chunked over the plane
        accp = ctx.enter_context(tc.tile_pool(name="acc", bufs=1, space="PSUM"))
        OUTv = OUT.rearrange("p (r c) -> p r c", c=HP)
        GPSET = frozenset((0, 2, 6, 8, 9, 11, 15, 17, 18, 20, 21, 23, 24, 26))
        for (c0, L), (r0, r1) in zip(CHUNKS, OUT_ROWS):
            den = accp.tile([128, 1536], f32, tag="den")
            num = accp.tile([128, 1536], f32, tag="num")
            for kv in range(27):
                kd, r = divmod(kv, 9)
                kh, kw = divmod(r, 3)
                dd = (kh - 1) * HP + (kw - 1)
                s_t = wpool.tile([128, 1536], f32, tag="s")
                nc.vector.scalar_tensor_tensor(
                    s_t[:, :L], Kp[kd][:, c0 + dd:c0 + dd + L],
                    b_s[:, kv:kv + 1], Q[:, c0:c0 + L], Alu.add, Alu.mult)
                e_t = wpool.tile([128, 1536], edt, tag="e")
                # bias keeps exp inside the ACT table range (softmax is
                # shift-invariant; the -28 cancels via the ln/exp normalize)
                nc.scalar.activation(e_t[:, :L], s_t[:, :L], Act.Exp, bias=ebias[:])
                ev_t = wpool.tile([128, 1536], edt, tag="ev")
                # DVE is the bottleneck engine; hand ~half the e*v products
                # to the otherwise-idle GPSIMD (stock Q7 tensor_tensor).
                ev_eng = nc.gpsimd if (kw == 1 or kv in GPSET) else nc.vector
                ev_eng.tensor_mul(ev_t[:, :L], e_t[:, :L],
                                  Vp[kd][:, c0 + dd:c0 + dd + L])
                st, sp = kv == 0, kv == 26
                for a, bl in _subs(L):
                    nc.tensor.matmul(den[:, a:a + bl], id_s[:], e_t[:, a:a + bl],
                                     start=st, stop=sp)
                    nc.tensor.matmul(num[:, a:a + bl], id_s[:], ev_t[:, a:a + bl],
                                     start=st, stop=sp)
            l_t = wpool.tile([128, 1536], f32, tag="s")
            nc.scalar.activation(l_t[:, :L], den[:, :L], Act.Ln)
            f_t = wpool.tile([128, 1536], f32, tag="f")
            nc.scalar.activation(f_t[:, :L], l_t[:, :L], Act.Exp, scale=-1.0)
            nc.vector.tensor_mul(OUT[:, c0:c0 + L], num[:, :L], f_t[:, :L])
            # rows fully covered by chunks <= this one stream out immediately
            nc.sync.dma_start(out_d[:, r0:r1, :],
                              OUTv[:, 1 + r0:1 + r1, 1:65])
    nc.finalize()
    return nc


def kernel(x, w_q, w_k, w_v, rel_d, rel_h, rel_w):
    from concourse.bass_utils import run_bass_kernel_spmd

    x = np.asarray(x, np.float32)
    rd = np.asarray(rel_d, np.float32).reshape(21, 3)
    rh = np.asarray(rel_h, np.float32).reshape(21, 3)
    rw = np.asarray(rel_w, np.float32).reshape(22, 3)

    xp = np.zeros((64, 18, HP, HP), np.float32)
    xp[:, 1:17, 1:65, 1:65] = x[0]

    B = np.zeros((128, 27), np.float32)
    for c in range(64):
        for kv in range(27):
            kd, r = divmod(kv, 9)
            kh, kw = divmod(r, 3)
            b = rd[c, kd] if c < 21 else (rh[c - 21, kh] if c < 42 else rw[c - 42, kw])
            B[c, kv] = B[64 + c, kv] = b

    idt = np.eye(128, dtype=np.float32)
    if E_BF16:
        import ml_dtypes
        idt = idt.astype(ml_dtypes.bfloat16)
    com = {
        "wk2": np.concatenate([w_k.T, w_k.T], 1).astype(np.float32).copy(),
        "wv2": np.concatenate([w_v.T, w_v.T], 1).astype(np.float32).copy(),
        "wq2": np.concatenate([w_q.T, w_q.T], 1).astype(np.float32).copy(),
        "bias": B, "ident": idt,
    }
    in_maps = []
    for i in range(8):
        m = dict(com)
        m["xs"] = xp[:, 2 * i:2 * i + 4].reshape(64, NPL * HW).copy()
        in_maps.append(m)

    if "nc" not in _CACHE:
        _CACHE["nc"] = _build()
    res = run_bass_kernel_spmd(_CACHE["nc"], in_maps, list(range(8)))

    out = np.empty((1, 64, 16, 64, 64), np.float32)
    for i in range(8):
        arr = res.results[i]["out"].reshape(2, 64, 64, 64)
        out[0, :, 2 * i] = arr[0]
        out[0, :, 2 * i + 1] = arr[1]
    return out



# revision 5
# speedup vs baseline: 1.5976x; 1.5976x over previous
"""AttentionConv3D Trainium2 kernel.

Computation (per channel c, voxel (d,h,w)):
    q,k,v = 1x1x1 convs of x;  s_kv = q * (k_pad[nbr kv] + rel_bias(c,kv))
    out   = sum_kv softmax_kv(s) * v_pad[nbr kv]         (27 = 3x3x3 window)

Strategy: depth-shard over 8 cores (2 output depth planes each, 1-plane halo).
Host zero-pads x to [64,18,66,66] so the channel-mix matmuls directly produce
zero-padded k/v/q planes. On-device layout: partition p = dl*64 + c
(dl in {0,1} local depth), free dim = padded 66x66 plane (4356).
Per kv-neighbor the window access is a free-dim offset (kh-1)*66 + (kw-1) into
one of three depth-plane buffers K[kd]; the rel bias collapses to a
per-partition scalar B[p, kv], so s = (K_shift + B)*q is ONE DVE
scalar_tensor_tensor op. exp on ACT; numerator/denominator accumulated with
identity matmuls into PSUM on the TensorEngine; 1/den via exp(-ln(den)) on ACT.

Host<->device transfer (the axon tunnel, ~60-90 MB/s) dominates wall time, so
all traffic is fp16 and packed into a single input tensor per core:
cols [0, 4*4356)      four padded x depth-planes
cols [17424, 17808)   wk/wv/wq dual-copy [64,128] each
cols [17808, 17835)   rel-bias half-table [64,27]
The 128x128 accumulation identity is built on-device (affine_select), the
output-buffer zeros are materialized in-graph, and the output returns as fp16.
The jitted PJRT executor is cached so repeat calls skip re-trace/re-jit.
"""

import sys
import numpy as np

for _p in ("/opt/trn_rl_repo", "/root/.axon_site/_ro/trn_rl_repo"):
    if _p not in sys.path:
        sys.path.insert(0, _p)

HP = 66               # padded plane edge
HW = HP * HP          # 4356
NPL = 4               # k/v depth planes per core (2 outputs + halo)
XCOLS = NPL * HW + 3 * 128 + 27   # packed input: planes | wk wv wq | bias
WOFF = NPL * HW       # 17424
BOFF = WOFF + 3 * 128
CHUNKS = [(67, 1402), (1469, 1536), (3005, 1284)]  # covers [67, 4289); chunk 0's
# window reads ([67-67, 67+67+1402) = [0,1536)) fit inside proj col-chunk 0 so
# the kv loop overlaps the tail of the projection phase.
PROJ = [(0, 1536), (1536, 1536), (3072, 1284)]     # proj psum chunks over 4356
OUT_ROWS = [(0, 21), (21, 44), (44, 64)]           # row bands DMA'd per chunk

_CACHE = {}


def _subs(L):
    return [(0, 512), (512, 512), (1024, L - 1024)]


def _build():
    from contextlib import ExitStack
    import concourse.bacc as bacc
    import concourse.tile as tile
    from concourse import mybir

    f32 = mybir.dt.float32
    f16 = mybir.dt.float16
    bf16 = mybir.dt.bfloat16
    Alu = mybir.AluOpType
    Act = mybir.ActivationFunctionType

    nc = bacc.Bacc("TRN2", target_bir_lowering=False)
    xs_d = nc.dram_tensor("xs", [64, XCOLS], f16, kind="ExternalInput")
    out_d = nc.dram_tensor("out", [128, 64, 64], f16, kind="ExternalOutput")

    with tile.TileContext(nc) as tc, ExitStack() as ctx:
        singles = ctx.enter_context(tc.tile_pool(name="singles", bufs=1))
        planes = ctx.enter_context(tc.tile_pool(name="planes", bufs=1))
        wpool = ctx.enter_context(tc.tile_pool(name="work", bufs=2))

        W = singles.tile([64, 3 * 128 + 27], f16, tag="w")
        nc.sync.dma_start(W[:], xs_d[:, WOFF:XCOLS])
        wk_s = W[:, 0:128]
        wv_s = W[:, 128:256]
        wq_s = W[:, 256:384]
        b16 = singles.tile([128, 27], f16, tag="b16")
        nc.sync.dma_start(b16[0:64, :], xs_d[:, BOFF:XCOLS])
        nc.sync.dma_start(b16[64:128, :], xs_d[:, BOFF:XCOLS])
        b_s = singles.tile([128, 27], f32, tag="b")
        nc.scalar.copy(b_s[:], b16[:])
        ebias = singles.tile([128, 1], f32, tag="ebias")
        nc.vector.memset(ebias[:], -28.0)
        id_s = singles.tile([128, 128], bf16, tag="id")
        nc.gpsimd.memset(id_s[:], 1.0)
        nc.gpsimd.affine_select(id_s[:], id_s[:], [[1, 128]], Alu.is_equal,
                                0.0, base=0, channel_multiplier=-1)

        Kp = [planes.tile([128, HW], f32, tag=f"k{i}", name=f"k{i}") for i in range(3)]
        Vp = [planes.tile([128, HW], bf16, tag=f"v{i}", name=f"v{i}") for i in range(3)]
        Q = planes.tile([128, HW], f32, tag="q")
        OUT = planes.tile([128, HW], f16, tag="o")

        # ---- projections: plane m of xs -> k/v (dual-copy weights give the
        # same output plane on partitions 0:64 and 64:128), q for m in {1,2}.
        # column-chunk OUTER so all planes' first 1536 columns (what kv chunk 0
        # needs) are projected before any plane's later columns.
        with tc.tile_pool(name="xp", bufs=3) as xpool, \
             tc.tile_pool(name="pp", bufs=2, space="PSUM") as ppool:
            for base, L3 in PROJ:
                for m in range(NPL):
                    X = xpool.tile([64, 1536], f16, tag="x")
                    nc.sync.dma_start(X[:, :L3],
                                      xs_d[:, m * HW + base:m * HW + base + L3])
                    projs = [(wk_s, "k"), (wv_s, "v")]
                    if m in (1, 2):
                        projs.append((wq_s, "q"))
                    for w_s, kind in projs:
                        pp = ppool.tile([128, 1536], f32, tag="pp")
                        for a, bl in _subs(L3):
                            nc.tensor.matmul(pp[:, a:a + bl], w_s,
                                             X[:, a:a + bl],
                                             start=True, stop=True)
                        sl = (slice(0, 64), slice(base, base + L3))
                        sh = (slice(64, 128), slice(base, base + L3))
                        if kind == "k":
                            # split k evacuations across DVE/ACT to keep DVE,
                            # the span-limiting engine, under ACT's load
                            if m <= 2:
                                nc.vector.tensor_copy(Kp[m][sl], pp[0:64, :L3])
                            if m >= 1:
                                nc.scalar.copy(Kp[m - 1][sh], pp[64:128, :L3])
                        elif kind == "v":
                            if m <= 2:
                                nc.scalar.copy(Vp[m][sl], pp[0:64, :L3])
                            if m >= 1:
                                nc.scalar.copy(Vp[m - 1][sh], pp[64:128, :L3])
                        elif m == 1:
                            nc.vector.tensor_copy(Q[sl], pp[0:64, :L3])
                        else:
                            nc.scalar.copy(Q[sh], pp[64:128, :L3])

        # ---- 27-neighbor softmax attention, PSUM-chunked over the plane
        accp = ctx.enter_context(tc.tile_pool(name="acc", bufs=1, space="PSUM"))
        OUTv = OUT.rearrange("p (r c) -> p r c", c=HP)
        GPSET = frozenset((0, 2, 6, 8, 9, 11, 15, 17, 18, 20, 21, 23, 24, 26))
        for (c0, L), (r0, r1) in zip(CHUNKS, OUT_ROWS):
            den = accp.tile([128, 1536], f32, tag="den")
            num = accp.tile([128, 1536], f32, tag="num")
            for kv in range(27):
                kd, r = divmod(kv, 9)
                kh, kw = divmod(r, 3)
                dd = (kh - 1) * HP + (kw - 1)
                s_t = wpool.tile([128, 1536], f32, tag="s")
                nc.vector.scalar_tensor_tensor(
                    s_t[:, :L], Kp[kd][:, c0 + dd:c0 + dd + L],
                    b_s[:, kv:kv + 1], Q[:, c0:c0 + L], Alu.add, Alu.mult)
                e_t = wpool.tile([128, 1536], bf16, tag="e")
                # bias keeps exp inside the ACT table range (softmax is
                # shift-invariant; the -28 cancels via the ln/exp normalize)
                nc.scalar.activation(e_t[:, :L], s_t[:, :L], Act.Exp, bias=ebias[:])
                ev_t = wpool.tile([128, 1536], bf16, tag="ev")
                # DVE is the bottleneck engine; hand ~half the e*v products
                # to the otherwise-idle GPSIMD (stock Q7 tensor_tensor).
                ev_eng = nc.gpsimd if (kw == 1 or kv in GPSET) else nc.vector
                ev_eng.tensor_mul(ev_t[:, :L], e_t[:, :L],
                                  Vp[kd][:, c0 + dd:c0 + dd + L])
                st, sp = kv == 0, kv == 26
                for a, bl in _subs(L):
                    nc.tensor.matmul(den[:, a:a + bl], id_s[:], e_t[:, a:a + bl],
                                     start=st, stop=sp)
                    nc.tensor.matmul(num[:, a:a + bl], id_s[:], ev_t[:, a:a + bl],
                                     start=st, stop=sp)
            l_t = wpool.tile([128, 1536], f32, tag="s")
            nc.scalar.activation(l_t[:, :L], den[:, :L], Act.Ln)
            f_t = wpool.tile([128, 1536], f32, tag="f")
            nc.scalar.activation(f_t[:, :L], l_t[:, :L], Act.Exp, scale=-1.0)
            nc.vector.tensor_mul(OUT[:, c0:c0 + L], num[:, :L], f_t[:, :L])
            # rows fully covered by chunks <= this one stream out immediately
            nc.sync.dma_start(out_d[:, r0:r1, :],
                              OUTv[:, 1 + r0:1 + r1, 1:65])
    nc.finalize()
    return nc


def _make_runner():
    import jax
    import jax.numpy as jnp
    from jax.sharding import Mesh, PartitionSpec
    from jax.experimental.shard_map import shard_map
    from concourse import mybir
    from concourse.bass2jax import (
        install_neuronx_cc_hook, partition_id_tensor, _bass_exec_p)

    nc = _build()
    install_neuronx_cc_hook()
    partition_name = (nc.partition_id_tensor.name
                      if nc.partition_id_tensor else None)
    in_names, out_names, out_avals = [], [], []
    for alloc in nc.m.functions[0].allocations:
        if not isinstance(alloc, mybir.MemoryLocationSet):
            continue
        name = alloc.memorylocations[0].name
        if alloc.kind == "ExternalInput":
            if name != partition_name:
                in_names.append(name)
        elif alloc.kind == "ExternalOutput":
            out_names.append(name)
            out_avals.append(jax.core.ShapedArray(
                tuple(alloc.tensor_shape), mybir.dt.np(alloc.dtype)))
    # out-named operands are omitted: the kernel writes every output element,
    # so no pre-zeroed donated buffers are needed (saves their host upload)
    all_names = tuple(in_names)
    if partition_name is not None:
        all_names = all_names + (partition_name,)

    def _body(*args):
        operands = list(args)
        if partition_name is not None:
            operands.append(partition_id_tensor())
        outs = _bass_exec_p.bind(
            *operands, out_avals=tuple(out_avals), in_names=all_names,
            out_names=tuple(out_names), lowering_input_output_aliases=(),
            sim_require_finite=True, sim_require_nnan=True, nc=nc)
        return tuple(outs)

    n_cores = 8
    devices = jax.devices()[:n_cores]
    mesh = Mesh(np.asarray(devices), ("core",))
    sharded = jax.jit(
        shard_map(_body, mesh=mesh,
                  in_specs=(PartitionSpec("core"),) * len(in_names),
                  out_specs=(PartitionSpec("core"),) * len(out_names),
                  check_rep=False),
        keep_unused=True)
    return sharded


def kernel(x, w_q, w_k, w_v, rel_d, rel_h, rel_w):
    x = np.asarray(x, np.float32)
    rd = np.asarray(rel_d, np.float32).reshape(21, 3)
    rh = np.asarray(rel_h, np.float32).reshape(21, 3)
    rw = np.asarray(rel_w, np.float32).reshape(22, 3)

    xp = np.zeros((64, 18, HP, HP), np.float16)
    xp[:, 1:17, 1:65, 1:65] = x[0]

    # rel bias half-table: rows = channel (dup'd to 64:128 on device)
    kvi = np.arange(27)
    Bh = np.empty((64, 27), np.float16)
    Bh[0:21] = rd[:, kvi // 9]
    Bh[21:42] = rh[:, (kvi % 9) // 3]
    Bh[42:64] = rw[:, kvi % 3]

    wpack = np.empty((64, 3 * 128 + 27), np.float16)
    wpack[:, 0:128] = np.concatenate([w_k.T, w_k.T], 1)
    wpack[:, 128:256] = np.concatenate([w_v.T, w_v.T], 1)
    wpack[:, 256:384] = np.concatenate([w_q.T, w_q.T], 1)
    wpack[:, 384:411] = Bh

    xs_all = np.empty((8 * 64, XCOLS), np.float16)
    for i in range(8):
        xs_all[64 * i:64 * i + 64, :WOFF] = \
            xp[:, 2 * i:2 * i + 4].reshape(64, WOFF)
        xs_all[64 * i:64 * i + 64, WOFF:] = wpack

    if "run" not in _CACHE:
        _CACHE["run"] = _make_runner()
    out_arrs = _CACHE["run"](xs_all)
    res = np.asarray(out_arrs[0])          # [8*128, 64, 64] fp16

    # partitions p = dl*64 + c; depth = 2*core + dl
    res = res.reshape(8, 2, 64, 64, 64)
    out = res.transpose(2, 0, 1, 3, 4).astype(np.float32)
    return np.ascontiguousarray(out.reshape(1, 64, 16, 64, 64))


# revision 9
# speedup vs baseline: 3.3191x; 2.0776x over previous
"""AttentionConv3D Trainium2 kernel.

Computation (per channel c, voxel (d,h,w)):
    q,k,v = 1x1x1 convs of x;  s_kv = q * (k_pad[nbr kv] + rel_bias(c,kv))
    out   = sum_kv softmax_kv(s) * v_pad[nbr kv]         (27 = 3x3x3 window)

Host<->device transfer over the axon tunnel (~50-90 MB/s) dominates wall
time, so the sharding/layout minimizes bytes moved:

H-shard over 8 cores: core i owns output rows 8i..8i+8 and receives the 10
padded H-rows 8i..8i+10 (1-row halo each side) of ALL 16 depth planes --
25% input overhead vs 100% for depth-sharding.  All traffic is fp16, packed
into one input tensor per core:
    cols [0, 16*10*WP)  x strip, n = d*(10*WP) + r*WP + wp  (WP = W+2 padded)
    then wk|wv|wq [64,64] each and rel-bias [64,27]
Output returns fp16 [64, 16*8*W] and is upcast on host.

On-device layout: partition p = channel (64), free dim = strip voxels.
K/V strips [18 planes, 10 rows, WP] f32/bf16 (depth-pad planes memset); the
1x1 convs project the already-zero-padded x so W/H pad cells come out zero,
matching the reference's pad-then-unfold semantics.  Per kv-neighbor the
window access is a free-dim offset (kd*660 + kh*66 + kw); the rel bias is a
per-partition scalar so s = (K_shift + B)*q is ONE DVE scalar_tensor_tensor
op.  exp on ACT (bias -28 keeps the table range; bf16 e/ev avoids fp16
underflow of exp(-28)); num/den accumulated with an on-device-built identity
matmul into PSUM; 1/den via exp(-ln(den)) on ACT.

The jitted PJRT executor is cached so repeat calls skip re-trace/re-jit, and
no zero output buffers are uploaded (the kernel writes every output element).
"""

import sys
import numpy as np

for _p in ("/opt/trn_rl_repo", "/root/.axon_site/_ro/trn_rl_repo"):
    if _p not in sys.path:
        sys.path.insert(0, _p)

NSPLIT = 1            # W-split pipelining factor (1 = single call)
D, H, W = 16, 64, 64
ROWS = 10             # strip rows per core: 8 output + 1 halo each side
_CACHE = {}


def _subs(L):
    return [(a, min(512, L - a)) for a in range(0, L, 512)]


def _build(wn):
    """Build the Bass program for output width wn (strip width wn+2)."""
    from contextlib import ExitStack
    import concourse.bacc as bacc
    import concourse.tile as tile
    from concourse import mybir

    wp = wn + 2                    # padded strip width
    pl = ROWS * wp                 # cols per (plane, strip): 10*wp
    xc = D * pl                    # x cols in the packed input
    on = 8 * wn                    # out cols per depth plane
    xcols = xc + 3 * 64 + 27

    f32 = mybir.dt.float32
    f16 = mybir.dt.float16
    bf16 = mybir.dt.bfloat16
    Alu = mybir.AluOpType
    Act = mybir.ActivationFunctionType

    nc = bacc.Bacc("TRN2", target_bir_lowering=False)
    xs_d = nc.dram_tensor("xs", [64, xcols], f16, kind="ExternalInput")
    out_d = nc.dram_tensor("out", [64, D * on], f16, kind="ExternalOutput")

    with tile.TileContext(nc) as tc, ExitStack() as ctx:
        singles = ctx.enter_context(tc.tile_pool(name="singles", bufs=1))
        planes = ctx.enter_context(tc.tile_pool(name="planes", bufs=1))
        wpool = ctx.enter_context(tc.tile_pool(name="work", bufs=2))

        Wt = singles.tile([64, 3 * 64 + 27], f16, tag="w")
        nc.sync.dma_start(Wt[:], xs_d[:, xc:xcols])
        wk_s = Wt[:, 0:64]
        wv_s = Wt[:, 64:128]
        wq_s = Wt[:, 128:192]
        b16 = Wt[:, 192:219]
        b_s = singles.tile([64, 27], f32, tag="b")
        nc.scalar.copy(b_s[:], b16)
        ebias = singles.tile([64, 1], f32, tag="ebias")
        nc.vector.memset(ebias[:], -28.0)
        id_s = singles.tile([64, 64], bf16, tag="id")
        nc.gpsimd.memset(id_s[:], 1.0)
        nc.gpsimd.affine_select(id_s[:], id_s[:], [[1, 64]], Alu.is_equal,
                                0.0, base=0, channel_multiplier=-1)

        # K/V strips: 18 depth planes (1 zero pad each side), 10 rows, wp cols
        Kt = planes.tile([64, (D + 2) * pl], f32, tag="k")
        Vt = planes.tile([64, (D + 2) * pl], bf16, tag="v")
        Q = planes.tile([64, D * on], f32, tag="q")
        OUT = planes.tile([64, D * on], f16, tag="o")
        nc.vector.memset(Kt[:, 0:pl], 0.0)
        nc.vector.memset(Kt[:, (D + 1) * pl:], 0.0)
        nc.gpsimd.memset(Vt[:, 0:pl], 0.0)
        nc.gpsimd.memset(Vt[:, (D + 1) * pl:], 0.0)

        X = planes.tile([64, xc], f16, tag="x")
        nc.sync.dma_start(X[:], xs_d[:, 0:xc])

        # ---- projections: one psum chunk per depth plane; the x strip is
        # already zero-padded so pad cells project to zero
        with tc.tile_pool(name="pp", bufs=2, space="PSUM") as ppool:
            for d in range(D):
                for w_s, kind in ((wk_s, "k"), (wv_s, "v"), (wq_s, "q")):
                    pp = ppool.tile([64, pl], f32, tag="pp")
                    for a, bl in _subs(pl):
                        nc.tensor.matmul(pp[:, a:a + bl], w_s,
                                         X[:, d * pl + a:d * pl + a + bl],
                                         start=True, stop=True)
                    dst = (d + 1) * pl
                    if kind == "k":
                        nc.vector.tensor_copy(Kt[:, dst:dst + pl], pp[:, :pl])
                    elif kind == "v":
                        nc.scalar.copy(Vt[:, dst:dst + pl], pp[:, :pl])
                    else:
                        # q: interior rows 1..8, cols 1..wn+1 only
                        nc.scalar.copy(
                            Q[:, d * on:(d + 1) * on].rearrange(
                                "p (r w) -> p r w", w=wn),
                            pp[:, :pl].rearrange(
                                "p (r w) -> p r w", w=wp)[:, 1:9, 1:wn + 1])

        # ---- 27-neighbor softmax attention, PSUM-chunked over depth planes
        accp = ctx.enter_context(tc.tile_pool(name="acc", bufs=1, space="PSUM"))
        Kv3 = Kt.rearrange("p (d r w) -> p d r w", r=ROWS, w=wp)
        Vv3 = Vt.rearrange("p (d r w) -> p d r w", r=ROWS, w=wp)
        GPSET = frozenset((0, 2, 6, 8, 9, 11, 15, 17, 18, 20, 21, 23, 24, 26))
        dchunks = [(d0, min(3, D - d0)) for d0 in range(0, D, 3)]
        for d0, nd in dchunks:
            L = nd * on
            den = accp.tile([64, 3 * 8 * 64], f32, tag="den")
            num = accp.tile([64, 3 * 8 * 64], f32, tag="num")
            for kv in range(27):
                kd, r = divmod(kv, 9)
                kh, kw = divmod(r, 3)
                # engine ops are limited to 3-D APs (partition + 2 free
                # dims), so depth planes get individual instructions
                s_t = wpool.tile([64, 3 * 8 * 64], f32, tag="s")
                for dl in range(nd):
                    nc.vector.scalar_tensor_tensor(
                        s_t[:, dl * on:(dl + 1) * on].rearrange(
                            "p (r w) -> p r w", w=wn),
                        Kv3[:, d0 + kd + dl, kh:kh + 8, kw:kw + wn],
                        b_s[:, kv:kv + 1],
                        Q[:, (d0 + dl) * on:(d0 + dl + 1) * on].rearrange(
                            "p (r w) -> p r w", w=wn),
                        Alu.add, Alu.mult)
                e_t = wpool.tile([64, 3 * 8 * 64], bf16, tag="e")
                # bias keeps exp inside the ACT table range (softmax is
                # shift-invariant; the -28 cancels via the ln/exp normalize)
                nc.scalar.activation(e_t[:, :L], s_t[:, :L], Act.Exp,
                                     bias=ebias[:])
                ev_t = wpool.tile([64, 3 * 8 * 64], bf16, tag="ev")
                # split e*v products between DVE and the otherwise-idle GPSIMD
                ev_eng = nc.gpsimd if (kw == 1 or kv in GPSET) else nc.vector
                for dl in range(nd):
                    ev_eng.tensor_mul(
                        ev_t[:, dl * on:(dl + 1) * on].rearrange(
                            "p (r w) -> p r w", w=wn),
                        e_t[:, dl * on:(dl + 1) * on].rearrange(
                            "p (r w) -> p r w", w=wn),
                        Vv3[:, d0 + kd + dl, kh:kh + 8, kw:kw + wn])
                st, sp = kv == 0, kv == 26
                for a, bl in _subs(L):
                    nc.tensor.matmul(den[:, a:a + bl], id_s[:],
                                     e_t[:, a:a + bl], start=st, stop=sp)
                    nc.tensor.matmul(num[:, a:a + bl], id_s[:],
                                     ev_t[:, a:a + bl], start=st, stop=sp)
            l_t = wpool.tile([64, 3 * 8 * 64], f32, tag="s")
            nc.scalar.activation(l_t[:, :L], den[:, :L], Act.Ln)
            f_t = wpool.tile([64, 3 * 8 * 64], f32, tag="f")
            nc.scalar.activation(f_t[:, :L], l_t[:, :L], Act.Exp, scale=-1.0)
            nc.vector.tensor_mul(OUT[:, d0 * on:d0 * on + L],
                                 num[:, :L], f_t[:, :L])
            nc.sync.dma_start(out_d[:, d0 * on:d0 * on + L],
                              OUT[:, d0 * on:d0 * on + L])
    nc.finalize()
    return nc


def _make_runner(wn):
    import jax
    from jax.sharding import Mesh, PartitionSpec
    from jax.experimental.shard_map import shard_map
    from concourse import mybir
    from concourse.bass2jax import (
        install_neuronx_cc_hook, partition_id_tensor, _bass_exec_p)

    nc = _build(wn)
    install_neuronx_cc_hook()
    partition_name = (nc.partition_id_tensor.name
                      if nc.partition_id_tensor else None)
    in_names, out_names, out_avals = [], [], []
    for alloc in nc.m.functions[0].allocations:
        if not isinstance(alloc, mybir.MemoryLocationSet):
            continue
        name = alloc.memorylocations[0].name
        if alloc.kind == "ExternalInput":
            if name != partition_name:
                in_names.append(name)
        elif alloc.kind == "ExternalOutput":
            out_names.append(name)
            out_avals.append(jax.core.ShapedArray(
                tuple(alloc.tensor_shape), mybir.dt.np(alloc.dtype)))
    # out-named operands are omitted: the kernel writes every output element,
    # so no pre-zeroed donated buffers are needed (saves their host upload)
    all_names = tuple(in_names)
    if partition_name is not None:
        all_names = all_names + (partition_name,)

    def _body(*args):
        operands = list(args)
        if partition_name is not None:
            operands.append(partition_id_tensor())
        outs = _bass_exec_p.bind(
            *operands, out_avals=tuple(out_avals), in_names=all_names,
            out_names=tuple(out_names), lowering_input_output_aliases=(),
            sim_require_finite=True, sim_require_nnan=True, nc=nc)
        return tuple(outs)

    n_cores = 8
    devices = jax.devices()[:n_cores]
    mesh = Mesh(np.asarray(devices), ("core",))
    sharded = jax.jit(
        shard_map(_body, mesh=mesh,
                  in_specs=(PartitionSpec("core"),) * len(in_names),
                  out_specs=(PartitionSpec("core"),) * len(out_names),
                  check_rep=False),
        keep_unused=True)
    return sharded


def kernel(x, w_q, w_k, w_v, rel_d, rel_h, rel_w):
    x = np.asarray(x, np.float32)
    rd = np.asarray(rel_d, np.float32).reshape(21, 3)
    rh = np.asarray(rel_h, np.float32).reshape(21, 3)
    rw = np.asarray(rel_w, np.float32).reshape(22, 3)

    wn = W // NSPLIT
    wp = wn + 2
    pl = ROWS * wp
    xc = D * pl
    xcols = xc + 3 * 64 + 27
    on = 8 * wn

    # rel bias table: rows = channel, cols = kv = kd*9+kh*3+kw
    kvi = np.arange(27)
    wpack = np.empty((64, 3 * 64 + 27), np.float16)
    wpack[:, 0:64] = w_k.T
    wpack[:, 64:128] = w_v.T
    wpack[:, 128:192] = w_q.T
    Bh = np.empty((64, 27), np.float16)
    Bh[0:21] = rd[:, kvi // 9]
    Bh[21:42] = rh[:, (kvi % 9) // 3]
    Bh[42:64] = rw[:, kvi % 3]
    wpack[:, 192:219] = Bh

    # globally padded x: [c, d, 66 rows, 66 cols]
    xr = np.zeros((64, D, H + 2, W + 2), np.float16)
    xr[:, :, 1:65, 1:65] = x[0]

    if "runs" not in _CACHE:
        _CACHE["runs"] = _make_runner(wn)

    outs = []
    for j in range(NSPLIT):
        xs_all = np.empty((8 * 64, xcols), np.float16)
        for i in range(8):
            xs_all[64 * i:64 * i + 64, :xc] = \
                xr[:, :, 8 * i:8 * i + ROWS,
                   j * wn:j * wn + wp].reshape(64, xc)
            xs_all[64 * i:64 * i + 64, xc:] = wpack
        outs.append(_CACHE["runs"](xs_all))

    full = np.empty((64, D, H, W), np.float32)
    for j in range(NSPLIT):
        res = np.asarray(outs[j][0])       # [8*64, D*8*wn] fp16
        res = res.reshape(8, 64, D, 8, wn)
        # out[c, d, 8i+r, j*wn + w] = res[i, c, d, r, w]
        full[:, :, :, j * wn:(j + 1) * wn] = \
            res.transpose(1, 2, 0, 3, 4).reshape(64, D, H, wn)
    return np.ascontiguousarray(full.reshape(1, 64, D, H, W))
